# revision 1
# baseline (speedup 1.0000x reference)
"""Trainium2 Bass kernel for nn_DecoderLayer (B=2, C=2048, M=1024, H=16, K=V=64, F=4096).

Sharding: row-parallel across 8 cores - 4 cores per batch, 512 query rows
each. K/V computed locally per row-slice, AllGathered within each batch's
4-core group (replica groups [0-3],[4-7]). No AllReduce.

Layout: activations feature-major; weights natural [in,out]. The softmax here
is ~argmax over N(0,1024) logits: noise entering the logits (or entering n1,
which feeds cross-attention Q) is amplified ~20x by near-tie blend rows. So
every path that reaches logits runs in split-bf16 (hi/lo pairs, ~2^-17):
Q/K/V projections are 3-pass (wh*xh + wh*xl + wl*xh), QK is 2-pass
(khi_aug x qhi_aug + [khi;klo] x [qlo;qhi]), self-attention V is 2-pass,
n1 is stored as a bf16 hi/lo pair. Non-cascading paths are cheap: E bf16,
cross-V bf16, preT/wo f32r, FFN f32r/bf16. Predicted rel err ~7e-3.

Attention per head:
  QK-B (hi only) -> logits [d, c] PSUM -> DVE reduce_max(negate) -> -max
  PE-transpose -> -max row [1, d] -> row 64 of the augmented hi Q operand
  QK-A (2-pass) -> (l - max) [c, d] -> ACT exp(scale=1/8) -> E bf16
  causal mask via per-core bf16 lambda tiles (one SPMD program, all cores)
  AV: [vhi|ones] x E (+ [vlo|0] x E for self) -> preT_aug; row 64 = sums
  1/sum via reciprocal + gpsimd partition_broadcast -> preT f32r -> wo f32r
  residual adds + feature-dim LN (ones-matmul stats + rank-1 A/B tiles)
"""
import os
import sys
import numpy as np

for _p in ("/opt/trn_rl_repo", "/root/.axon_site/_ro/trn_rl_repo"):
    if os.path.isdir(_p) and _p not in sys.path:
        sys.path.insert(0, _p)

import ml_dtypes
import concourse.bass as bass
import concourse.tile as tile
from concourse import bacc, mybir
from concourse.masks import make_identity

F32 = mybir.dt.float32
F32R = mybir.dt.float32r
BF16 = mybir.dt.bfloat16
AF = mybir.ActivationFunctionType
ALU = mybir.AluOpType
AX = mybir.AxisListType

B, C, M, H, K, V, F = 2, 2048, 1024, 16, 64, 64, 4096
EPS = 1e-5
SCALE = 1.0 / 8.0
MASKB = -100.0 * SCALE
NCORES = 8
CPB = NCORES // B
D = C // CPB
NH2 = H // 2
MT = M // 128
CT = C // 128
DT = D // 128
FT = F // 128
VA = V + 1
RG = [[0, 1, 2, 3], [4, 5, 6, 7]]


def _stream(nc, pool, d_h, r0, c0, rows, cols, dt, tag, bufs):
    w = pool.tile([rows, cols], dt, tag=tag, bufs=bufs, name=tag)
    nc.sync.dma_start(out=w[:], in_=d_h.ap()[r0:r0 + rows, c0:c0 + cols])
    return w


def _proj3(nc, sbp, psp, wh_d, wl_d, src, tag, dst_dram=None, keep_bufs=2,
           jblock=4):
    """3-pass split-bf16 projection: out = wh.T(xh+xl) + wl.T xh.

    src: ("dram", hi_handle, lo_handle) or ("tiles", hi_list, lo_list).
    Returns (hi, lo) bf16 tile lists; optionally DMAs to dst_dram rows
    j*128 (hi) / M + j*128 (lo).
    """
    his, los = [None] * MT, [None] * MT
    for jb in range(0, MT, jblock):
        pos = [psp.tile([128, D], F32, tag="ps", bufs=8, name="po")
               for _ in range(jblock)]
        for mt in range(MT):
            if src[0] == "dram":
                xh = _stream(nc, sbp, src[1], mt * 128, 0, 128, D, BF16,
                             "prj_xh", 2)
                xl = _stream(nc, sbp, src[2], mt * 128, 0, 128, D, BF16,
                             "prj_xl", 2)
            else:
                xh, xl = src[1][mt], src[2][mt]
            for ji in range(jblock):
                j = jb + ji
                wh = _stream(nc, sbp, wh_d, mt * 128, j * 128, 128, 128,
                             BF16, "prj_wh", 3)
                wl = _stream(nc, sbp, wl_d, mt * 128, j * 128, 128, 128,
                             BF16, "prj_wl", 3)
                nc.tensor.matmul(pos[ji][:], wh[:], xh[:],
                                 start=(mt == 0), stop=False)
                nc.tensor.matmul(pos[ji][:], wh[:], xl[:],
                                 start=False, stop=False)
                nc.tensor.matmul(pos[ji][:], wl[:], xh[:],
                                 start=False, stop=(mt == MT - 1))
        for ji in range(jblock):
            j = jb + ji
            hi = sbp.tile([128, D], BF16, tag=tag + "_hi", bufs=keep_bufs,
                          name="hi")
            nc.vector.tensor_copy(hi[:], pos[ji][:])
            lo = sbp.tile([128, D], BF16, tag=tag + "_lo", bufs=keep_bufs,
                          name="lo")
            nc.vector.tensor_tensor(out=lo[:], in0=pos[ji][:], in1=hi[:],
                                    op=ALU.subtract)
            his[j], los[j] = hi, lo
            if dst_dram is not None:
                nc.sync.dma_start(out=dst_dram[j * 128:(j + 1) * 128, :],
                                  in_=hi[:])
                nc.sync.dma_start(
                    out=dst_dram[M + j * 128:M + (j + 1) * 128, :], in_=lo[:])
    return his, los


def _vproj3(nc, sbp, psp, wh_d, wl_d, srch_d, srcl_d, v_local, split_out,
            onesb, zerob):
    """Token-major V projection (3-pass split); writes v_local DRAM with
    interleaved ones columns (hi half) and zero columns (lo half)."""
    nhalf = 2 if split_out else 1
    for ctl in range(D // 128):
        for g in range(4):
            po = psp.tile([128, 256], F32, tag="ps", bufs=8, name="vpo")
            for mt in range(MT):
                xh = sbp.tile([128, 128], BF16, tag="vp_xh", bufs=2,
                              name="xh")
                nc.sync.dma_start(
                    out=xh[:],
                    in_=srch_d.ap().rearrange("m (ct p) -> m ct p", p=128)
                    [mt * 128:(mt + 1) * 128, ctl, :])
                xl = sbp.tile([128, 128], BF16, tag="vp_xl", bufs=2,
                              name="xl")
                nc.sync.dma_start(
                    out=xl[:],
                    in_=srcl_d.ap().rearrange("m (ct p) -> m ct p", p=128)
                    [mt * 128:(mt + 1) * 128, ctl, :])
                wh = _stream(nc, sbp, wh_d, mt * 128, g * 256, 128, 256,
                             BF16, "vp_wh", 3)
                wl = _stream(nc, sbp, wl_d, mt * 128, g * 256, 128, 256,
                             BF16, "vp_wl", 3)
                nc.tensor.matmul(po[:], xh[:], wh[:],
                                 start=(mt == 0), stop=False)
                nc.tensor.matmul(po[:], xl[:], wh[:], start=False, stop=False)
                nc.tensor.matmul(po[:], xh[:], wl[:], start=False,
                                 stop=(mt == MT - 1))
            vbh = sbp.tile([128, 256], BF16, tag="vp_o", bufs=2, name="vbh")
            nc.vector.tensor_copy(vbh[:], po[:])
            halves = [(0, vbh)]
            if split_out:
                vbl = sbp.tile([128, 256], BF16, tag="vp_l", bufs=2,
                               name="vbl")
                nc.vector.tensor_tensor(out=vbl[:], in0=po[:], in1=vbh[:],
                                        op=ALU.subtract)
                halves.append((1, vbl))
            for half, vb in halves:
                nc.sync.dma_start(
                    out=v_local[ctl * 128:(ctl + 1) * 128,
                                half * H * VA:(half + 1) * H * VA]
                    .rearrange("p (h w) -> p h w", h=H)
                    [:, 4 * g:4 * g + 4, 0:V],
                    in_=vb[:].rearrange("p (h w) -> p h w", h=4))
        for half, ob in ((0, onesb), (1, zerob))[:nhalf]:
            nc.sync.dma_start(
                out=v_local[ctl * 128:(ctl + 1) * 128,
                            half * H * VA:(half + 1) * H * VA]
                .rearrange("p (h w) -> p h w", h=H)[:, :, V:VA],
                in_=ob[:].rearrange("p (h o) -> p h o", o=1))


def _layernorm(nc, sbp, psp, consts, r_tiles, g_d, b_d, tag, out_bufs=8,
               split_out=False):
    """LN over the partition(feature) dim of 8 [128, D] f32 tiles."""
    ones128 = consts["ones128"]
    onesrow = consts["ones_row_f32"]
    gvec = sbp.tile([1, M], F32, tag="gbrow", bufs=2, name="gvec")
    nc.sync.dma_start(out=gvec[:], in_=g_d.ap())
    bvec = sbp.tile([1, M], F32, tag="gbrow", bufs=2, name="bvec")
    nc.sync.dma_start(out=bvec[:], in_=b_d.ap())
    pss = psp.tile([1, D], F32, tag="ps", bufs=8, name="pss")
    for mt in range(MT):
        nc.tensor.matmul(pss[:], ones128[:], r_tiles[mt][:],
                         start=(mt == 0), stop=(mt == MT - 1))
    psq = psp.tile([1, D], F32, tag="ps", bufs=8, name="psq")
    for mt in range(MT):
        sq = sbp.tile([128, D], F32, tag="lnsq", bufs=1, name="sq")
        nc.scalar.activation(out=sq[:], in_=r_tiles[mt][:], func=AF.Square)
        nc.tensor.matmul(psq[:], ones128[:], sq[:],
                         start=(mt == 0), stop=(mt == MT - 1))
    mu = sbp.tile([1, D], F32, tag="lnrow", bufs=5, name="mu")
    nc.vector.tensor_scalar_mul(mu[:], pss[:], 1.0 / M)
    var = sbp.tile([1, D], F32, tag="lnrow", bufs=5, name="var")
    nc.vector.tensor_scalar_mul(var[:], psq[:], 1.0 / M)
    mu2 = sbp.tile([1, D], F32, tag="lnrow", bufs=5, name="mu2")
    nc.vector.tensor_tensor(out=mu2[:], in0=mu[:], in1=mu[:], op=ALU.mult)
    nc.vector.tensor_tensor(out=var[:], in0=var[:], in1=mu2[:],
                            op=ALU.subtract)
    std = sbp.tile([1, D], F32, tag="lnrow", bufs=5, name="std")
    nc.scalar.activation(out=std[:], in_=var[:], func=AF.Sqrt,
                         bias=consts["eps1"][:])
    rstd = sbp.tile([1, D], F32, tag="lnrow", bufs=5, name="rstd")
    nc.vector.reciprocal(rstd[:], std[:])
    nmr = sbp.tile([1, D], F32, tag="lnrow", bufs=5, name="nmr")
    nc.vector.tensor_tensor(out=nmr[:], in0=mu[:], in1=rstd[:], op=ALU.mult)
    nc.vector.tensor_scalar_mul(nmr[:], nmr[:], -1.0)

    outs = []
    for mt in range(MT):
        g = gvec[:, mt * 128:(mt + 1) * 128]
        b = bvec[:, mt * 128:(mt + 1) * 128]
        pa = psp.tile([128, D], F32, tag="ps", bufs=8, name="pa")
        nc.tensor.matmul(pa[:], g, rstd[:], start=True, stop=True)
        pb = psp.tile([128, D], F32, tag="ps", bufs=8, name="pb")
        nc.tensor.matmul(pb[:], b, onesrow[:], start=True, stop=False)
        nc.tensor.matmul(pb[:], g, nmr[:], start=False, stop=True)
        tmp = sbp.tile([128, D], F32, tag="lntmp", bufs=2, name="tmp")
        nc.vector.tensor_tensor(out=tmp[:], in0=r_tiles[mt][:], in1=pa[:],
                                op=ALU.mult)
        if split_out:
            n32 = sbp.tile([128, D], F32, tag="lnn32", bufs=2, name="n32")
            nc.vector.tensor_tensor(out=n32[:], in0=tmp[:], in1=pb[:],
                                    op=ALU.add)
            nh = sbp.tile([128, D], BF16, tag=tag + "_h", bufs=out_bufs,
                          name="nh")
            nc.vector.tensor_copy(nh[:], n32[:])
            nl = sbp.tile([128, D], BF16, tag=tag + "_l", bufs=out_bufs,
                          name="nl")
            nc.vector.tensor_tensor(out=nl[:], in0=n32[:], in1=nh[:],
                                    op=ALU.subtract)
            outs.append((nh, nl))
        else:
            n = sbp.tile([128, D], F32R, tag=tag, bufs=out_bufs, name="n")
            nc.vector.tensor_tensor(out=n[:], in0=tmp[:], in1=pb[:],
                                    op=ALU.add)
            outs.append(n)
    return outs


def _attention(nc, sbp, psp, consts, qhi, qlo, k_full, v_full, masked,
               vsplit, wo_d, xres_fn, g_d, b_d, ntag, mask_d, ln_split):
    """One attention block + residual + LN."""
    preT = [sbp.tile([128, D], F32R, tag="preT", bufs=8, name=f"preT{_j}")
            for _j in range(NH2)]

    for h in range(H):
        khi = sbp.tile([VA, C], BF16, tag="khi", bufs=2, name="khi")
        for r in range(CPB):
            nc.sync.dma_start(
                out=khi[0:K, r * D:(r + 1) * D],
                in_=k_full[r * 2 * M + h * K: r * 2 * M + h * K + K, :])
        nc.sync.dma_start(out=khi[K:VA, :], in_=consts["ones2048"].ap())
        kst = sbp.tile([128, C], BF16, tag="kst", bufs=2, name="kst")
        for r in range(CPB):
            nc.sync.dma_start(
                out=kst[0:K, r * D:(r + 1) * D],
                in_=k_full[r * 2 * M + h * K: r * 2 * M + h * K + K, :])
            nc.sync.dma_start(
                out=kst[K:128, r * D:(r + 1) * D],
                in_=k_full[r * 2 * M + M + h * K: r * 2 * M + M + h * K + K,
                           :])

        j2, o2 = h // 2, (h % 2) * K
        rhi = sbp.tile([VA, D], BF16, tag="rhi", bufs=2, name="rhi")
        nc.vector.tensor_copy(rhi[0:K, :], qhi[j2][o2:o2 + K, :])
        rst = sbp.tile([128, D], BF16, tag="rst", bufs=2, name="rst")
        nc.vector.tensor_copy(rst[0:K, :], qlo[j2][o2:o2 + K, :])
        nc.vector.tensor_copy(rst[K:128, :], qhi[j2][o2:o2 + K, :])

        # QK-B on hi: negated max per d-tile
        psm = psp.tile([1, D], F32, tag="ps", bufs=8, name="psm")
        for dt in range(DT):
            nm = sbp.tile([128, 1], F32, tag="nmax", bufs=4, name="nm")
            for cc in range(C // 512):
                psb = psp.tile([128, 512], F32, tag="ps", bufs=8, name="psb")
                nc.tensor.matmul(
                    psb[:], rhi[0:K, dt * 128:(dt + 1) * 128],
                    khi[0:K, cc * 512:(cc + 1) * 512],
                    start=True, stop=True)
                if cc == 0:
                    nc.vector.reduce_max(nm[:], psb[:], axis=AX.X, negate=True)
                else:
                    nm2 = sbp.tile([128, 1], F32, tag="nmax2", bufs=2,
                                   name="nm2")
                    nc.vector.reduce_max(nm2[:], psb[:], axis=AX.X,
                                         negate=True)
                    nc.vector.tensor_tensor(out=nm[:], in0=nm[:], in1=nm2[:],
                                            op=ALU.min)
            nc.tensor.transpose(psm[0:1, dt * 128:(dt + 1) * 128], nm[:],
                                consts["identb"][:])
        nc.vector.tensor_copy(rhi[K:VA, :], psm[:])

        # QK-A (2 passes) + exp (+mask) + AV
        pp = psp.tile([VA, D], F32, tag="ps", bufs=8, name="pp")
        for ct in range(CT):
            psa = psp.tile([128, D], F32, tag="ps", bufs=8, name="psa")
            nc.tensor.matmul(psa[:], khi[:, ct * 128:(ct + 1) * 128], rhi[:],
                             start=True, stop=False)
            nc.tensor.matmul(psa[:], kst[:, ct * 128:(ct + 1) * 128], rst[:],
                             start=False, stop=True)
            e = sbp.tile([128, D], BF16, tag="etile", bufs=3, name="e")
            nc.scalar.activation(out=e[:], in_=psa[:], func=AF.Exp,
                                 scale=SCALE)
            if masked:
                mt_ = sbp.tile([128, D], BF16, tag="mtile", bufs=1, name="mt_")
                nc.sync.dma_start(out=mt_[:], in_=mask_d.ap()[:, ct, :])
                nc.vector.tensor_tensor(out=e[:], in0=e[:], in1=mt_[:],
                                        op=ALU.mult)
            vhi = sbp.tile([128, VA], BF16, tag="vhi", bufs=3, name="vhi")
            nc.sync.dma_start(
                out=vhi[:],
                in_=v_full[ct * 128:(ct + 1) * 128, h * VA:(h + 1) * VA])
            nc.tensor.matmul(pp[:], vhi[:], e[:],
                             start=(ct == 0),
                             stop=(not vsplit and ct == CT - 1))
            if vsplit:
                vlo = sbp.tile([128, VA], BF16, tag="vlo", bufs=3, name="vlo")
                nc.sync.dma_start(
                    out=vlo[:],
                    in_=v_full[ct * 128:(ct + 1) * 128,
                               H * VA + h * VA: H * VA + (h + 1) * VA])
                nc.tensor.matmul(pp[:], vlo[:], e[:],
                                 start=False, stop=(ct == CT - 1))

        rec = sbp.tile([1, D], F32, tag="rec", bufs=1, name="rec")
        nc.vector.reciprocal(rec[:], pp[K:VA, :])
        rb = sbp.tile([K, D], F32, tag="rbcast", bufs=2, name="rb")
        nc.gpsimd.partition_broadcast(rb[:], rec[:])
        nc.vector.tensor_tensor(out=preT[j2][o2:o2 + K, :],
                                in0=pp[0:K, :], in1=rb[:], op=ALU.mult)

    # wo projection + residual
    r_tiles = []
    for mt in range(MT):
        po = psp.tile([128, D], F32, tag="ps", bufs=8, name="po")
        for j in range(NH2):
            w = _stream(nc, sbp, wo_d, j * 128, mt * 128, 128, 128, F32R,
                        "wo_w", 3)
            nc.tensor.matmul(po[:], w[:], preT[j][:],
                             start=(j == 0), stop=(j == NH2 - 1))
        adds = xres_fn(mt)
        r = sbp.tile([128, D], F32, tag="rres", bufs=8, name="r")
        nc.vector.tensor_tensor(out=r[:], in0=po[:], in1=adds[0][:],
                                op=ALU.add)
        for extra in adds[1:]:
            nc.vector.tensor_tensor(out=r[:], in0=r[:], in1=extra[:],
                                    op=ALU.add)
        r_tiles.append(r)

    n = _layernorm(nc, sbp, psp, consts, r_tiles, g_d, b_d, ntag,
                   split_out=ln_split)
    return n, r_tiles


def build():
    nc = bacc.Bacc("TRN2", target_bir_lowering=False, debug=False,
                   num_devices=NCORES)
    inp = {}

    def di(name, shape, dt):
        inp[name] = nc.dram_tensor(name, shape, dt, kind="ExternalInput")
        return inp[name]

    for nm in ("xh", "xl", "eh", "el"):
        di(nm, [M, D], BF16)
    for w in ("wq1", "wk1", "wv1", "wq2", "wk2", "wv2"):
        di(w + "h", [M, M], BF16)
        di(w + "l", [M, M], BF16)
    di("wo1", [M, M], F32R)
    di("wo2", [M, M], F32R)
    di("fw1", [M, F], F32R)
    di("fw2", [F, M], BF16)
    for v in ("g1", "b1", "g2", "b2", "g3", "b3"):
        di(v, [1, M], F32)
    di("fb1", [1, F], F32)
    di("fb2", [1, M], F32)
    di("ones2048", [1, C], BF16)
    di("maskT", [128, CT, D], BF16)
    outT = nc.dram_tensor("outT", [M, D], F32, kind="ExternalOutput")

    with tile.TileContext(nc) as tc:
        import contextlib
        with contextlib.ExitStack() as ctx:
            sbp = ctx.enter_context(tc.tile_pool(name="sb", bufs=1))
            psp = ctx.enter_context(tc.tile_pool(name="ps", bufs=1,
                                                 space="PSUM"))
            sing = ctx.enter_context(tc.tile_pool(name="sing", bufs=1))
            dram = ctx.enter_context(tc.tile_pool(name="dram", bufs=1,
                                                  space="DRAM"))

            consts = {}
            identb = sing.tile([128, 128], F32)
            make_identity(nc, identb[:])
            consts["identb"] = identb
            ones128 = sing.tile([128, 1], F32)
            nc.vector.memset(ones128[:], 1.0)
            consts["ones128"] = ones128
            onesrowf = sing.tile([1, D], F32)
            nc.vector.memset(onesrowf[:], 1.0)
            consts["ones_row_f32"] = onesrowf
            eps1 = sing.tile([1, 1], F32)
            nc.vector.memset(eps1[:], EPS)
            consts["eps1"] = eps1
            onesb = sing.tile([128, H], BF16)
            nc.vector.memset(onesb[:], 1.0)
            zerob = sing.tile([128, H], BF16)
            nc.vector.memset(zerob[:], 0.0)
            consts["ones2048"] = inp["ones2048"]
            fb1 = sing.tile([128, FT], F32)
            nc.sync.dma_start(
                out=fb1[:],
                in_=inp["fb1"].ap().rearrange("o (a b) -> o a b", b=128)[0]
                .rearrange("a b -> b a"))
            fb2c = sing.tile([128, MT], F32)
            nc.sync.dma_start(
                out=fb2c[:],
                in_=inp["fb2"].ap().rearrange("o (a b) -> o a b", b=128)[0]
                .rearrange("a b -> b a"))

            k1_local = dram.tile([2 * M, D], BF16)
            k1_full = dram.tile([CPB * 2 * M, D], BF16)
            v1_local = dram.tile([D, 2 * H * VA], BF16)
            v1_full = dram.tile([C, 2 * H * VA], BF16)
            k2_local = dram.tile([2 * M, D], BF16)
            k2_full = dram.tile([CPB * 2 * M, D], BF16)
            v2_local = dram.tile([D, H * VA], BF16)
            v2_full = dram.tile([C, H * VA], BF16)

            xsrc = ("dram", inp["xh"], inp["xl"])
            esrc = ("dram", inp["eh"], inp["el"])

            _proj3(nc, sbp, psp, inp["wk1h"], inp["wk1l"], xsrc, "k1",
                   dst_dram=k1_local)
            _vproj3(nc, sbp, psp, inp["wv1h"], inp["wv1l"], inp["xh"],
                    inp["xl"], v1_local, True, onesb, zerob)
            nc.gpsimd.collective_compute(
                "AllGather", ALU.bypass, replica_groups=RG,
                ins=[k1_local.opt()], outs=[k1_full.opt()])
            nc.gpsimd.collective_compute(
                "AllGather", ALU.bypass, replica_groups=RG,
                ins=[v1_local.opt()], outs=[v1_full.opt()])
            _proj3(nc, sbp, psp, inp["wk2h"], inp["wk2l"], esrc, "k2",
                   dst_dram=k2_local)
            _vproj3(nc, sbp, psp, inp["wv2h"], inp["wv2l"], inp["eh"],
                    inp["el"], v2_local, False, onesb, zerob)
            nc.gpsimd.collective_compute(
                "AllGather", ALU.bypass, replica_groups=RG,
                ins=[k2_local.opt()], outs=[k2_full.opt()])
            nc.gpsimd.collective_compute(
                "AllGather", ALU.bypass, replica_groups=RG,
                ins=[v2_local.opt()], outs=[v2_full.opt()])

            q1h, q1l = _proj3(nc, sbp, psp, inp["wq1h"], inp["wq1l"], xsrc,
                              "q", keep_bufs=8)

            def xres1(mt):
                a = _stream(nc, sbp, inp["xh"], mt * 128, 0, 128, D, BF16,
                            "xres", 2)
                bb = _stream(nc, sbp, inp["xl"], mt * 128, 0, 128, D, BF16,
                             "xres", 2)
                return [a, bb]

            n1, _ = _attention(nc, sbp, psp, consts, q1h, q1l, k1_full,
                               v1_full, True, True, inp["wo1"], xres1,
                               inp["g1"], inp["b1"], "nA", inp["maskT"],
                               ln_split=True)
            n1h = [t[0] for t in n1]
            n1l = [t[1] for t in n1]

            q2h, q2l = _proj3(nc, sbp, psp, inp["wq2h"], inp["wq2l"],
                              ("tiles", n1h, n1l), "q", keep_bufs=8)

            def xres2(mt):
                return [n1h[mt], n1l[mt]]

            n2, _ = _attention(nc, sbp, psp, consts, q2h, q2l, k2_full,
                               v2_full, False, False, inp["wo2"], xres2,
                               inp["g2"], inp["b2"], "nB", None,
                               ln_split=False)

            # FFN
            h1 = []
            for ft in range(FT):
                po = psp.tile([128, D], F32, tag="ps", bufs=8, name="fpo")
                for mt in range(MT):
                    w1 = _stream(nc, sbp, inp["fw1"], mt * 128, ft * 128,
                                 128, 128, F32R, "ffn1_w", 3)
                    nc.tensor.matmul(po[:], w1[:], n2[mt][:],
                                     start=(mt == 0), stop=(mt == MT - 1))
                o = sbp.tile([128, D], BF16, tag="h1", bufs=32, name="o")
                nc.scalar.activation(out=o[:], in_=po[:], func=AF.Relu,
                                     bias=fb1[:, ft:ft + 1])
                h1.append(o)

            r3 = []
            for mt in range(MT):
                po = psp.tile([128, D], F32, tag="ps", bufs=8, name="fpo2")
                for ft in range(FT):
                    w2 = _stream(nc, sbp, inp["fw2"], ft * 128, mt * 128,
                                 128, 128, BF16, "ffn2_w", 3)
                    nc.tensor.matmul(po[:], w2[:], h1[ft][:],
                                     start=(ft == 0), stop=(ft == FT - 1))
                t = sbp.tile([128, D], F32, tag="ffn_b", bufs=2, name="t")
                nc.scalar.activation(out=t[:], in_=po[:], func=AF.Identity,
                                     bias=fb2c[:, mt:mt + 1], scale=1.0)
                r = sbp.tile([128, D], F32, tag="rres", bufs=8, name="r")
                nc.vector.tensor_tensor(out=r[:], in0=t[:], in1=n2[mt][:],
                                        op=ALU.add)
                r3.append(r)

            n3 = _layernorm(nc, sbp, psp, consts, r3, inp["g3"], inp["b3"],
                            "nC", out_bufs=2)
            for mt in range(MT):
                nc.sync.dma_start(out=outT.ap()[mt * 128:(mt + 1) * 128, :],
                                  in_=n3[mt][:].bitcast(F32))

    nc.compile()
    return nc


_CACHE = {}


def _get_nc():
    if "nc" not in _CACHE:
        _CACHE["nc"] = build()
    return _CACHE["nc"]


def _make_maskT(q):
    lam = np.exp(np.float32(MASKB))
    D0 = q * D
    i = np.arange(128)[:, None]
    j = np.arange(D)[None, :]
    m = np.ones((128, CT, D), np.float32)
    for ct in range(CT):
        m[:, ct, :] = np.where(D0 + j >= ct * 128 + i, lam, 1.0)
    return m.astype(ml_dtypes.bfloat16)


def _split(a):
    hi = a.astype(ml_dtypes.bfloat16)
    lo = (a - hi.astype(np.float32)).astype(ml_dtypes.bfloat16)
    return np.ascontiguousarray(hi), np.ascontiguousarray(lo)


def make_in_maps(inputs):
    x = np.asarray(inputs["x"], np.float32)
    enc = np.asarray(inputs["enc_out"], np.float32)

    def packw(w):  # [H, M, K] -> [M, H*K]
        return np.ascontiguousarray(
            np.asarray(w, np.float32).transpose(1, 0, 2).reshape(M, -1))

    base = {
        "wo1": np.ascontiguousarray(
            np.asarray(inputs["wo1"], np.float32).reshape(H * V, M)),
        "wo2": np.ascontiguousarray(
            np.asarray(inputs["wo2"], np.float32).reshape(H * V, M)),
        "fw1": np.ascontiguousarray(np.asarray(inputs["fw1"], np.float32)),
        "fw2": np.asarray(inputs["fw2"], np.float32).astype(
            ml_dtypes.bfloat16),
        "g1": np.asarray(inputs["g1"], np.float32).reshape(1, M),
        "b1": np.asarray(inputs["b1"], np.float32).reshape(1, M),
        "g2": np.asarray(inputs["g2"], np.float32).reshape(1, M),
        "b2": np.asarray(inputs["b2"], np.float32).reshape(1, M),
        "g3": np.asarray(inputs["g3"], np.float32).reshape(1, M),
        "b3": np.asarray(inputs["b3"], np.float32).reshape(1, M),
        "fb1": np.asarray(inputs["fb1"], np.float32).reshape(1, F),
        "fb2": np.asarray(inputs["fb2"], np.float32).reshape(1, M),
        "ones2048": np.ones((1, C), ml_dtypes.bfloat16),
    }
    for w in ("wq1", "wk1", "wv1", "wq2", "wk2", "wv2"):
        hi, lo = _split(packw(inputs[w]))
        base[w + "h"] = hi
        base[w + "l"] = lo
    masks = [_make_maskT(q) for q in range(CPB)]
    in_maps = []
    for core in range(NCORES):
        b, q = core // CPB, core % CPB
        m = dict(base)
        xh, xl = _split(np.ascontiguousarray(x[b, q * D:(q + 1) * D, :].T))
        eh, el = _split(np.ascontiguousarray(enc[b, q * D:(q + 1) * D, :].T))
        m["xh"], m["xl"], m["eh"], m["el"] = xh, xl, eh, el
        m["maskT"] = masks[q]
        in_maps.append(m)
    return in_maps


def run_spmd(inputs, **kw):
    from concourse.bass_utils import run_bass_kernel_spmd
    nc = _get_nc()
    in_maps = make_in_maps(inputs)
    res = run_bass_kernel_spmd(nc, in_maps, core_ids=list(range(NCORES)), **kw)
    out = np.empty((B, C, M), np.float32)
    for core in range(NCORES):
        b, q = core // CPB, core % CPB
        out[b, q * D:(q + 1) * D, :] = res.results[core]["outT"].T
    return out, res


def kernel(**inputs):
    out, _ = run_spmd(inputs)
    return out



# revision 49
# speedup vs baseline: 1.8342x; 1.8342x over previous
"""Trainium2 Bass kernel for nn_DecoderLayer (B=2, C=2048, M=1024, H=16, K=V=64, F=4096).

Sharding: row-parallel across 8 cores - 4 cores per batch, 512 query rows
each. K/V computed locally per row-slice, AllGathered within each batch's
4-core group (replica groups [0-3],[4-7]). No AllReduce.

Numerics are identical to the validated baseline (split-bf16 hi/lo pairs on
every path that reaches softmax logits; see the error analysis there), but
the schedule is restructured for throughput. The baseline spent 60% of its
3.9ms wall time serially dispatching ~3.8k small DMAs on the Sync queue and
ran LN matmuls at the fp32 4-cycle/row rate. Here:
  - x/enc activation tiles are DMAed once and reused (proj moving operand,
    vproj stationary, residuals).
  - weights stream as [128,512] slabs (1-2KB per partition line).
  - K/V DRAM layouts are arranged so each attention head loads K with 3
    DMAs ([kh;kl] x [c], plus a ones row) and V with one slab DMA per
    4-head group.
  - all LayerNorm matmul moving operands are float32r (1 cycle/row at
    free-dim 512 vs 4 for plain fp32).
  - PSUM is budgeted exactly: pacc(4) transient/chains + acc(2) attention
    accumulators + psm(2) max-rows = 8 banks.

Attention per head (unchanged math):
  QK-B (hi only) -> logits [d, c] PSUM -> DVE reduce_max(negate) -> -max
  PE-transpose -> -max row -> row 64 of the augmented hi Q operand
  QK-A (2-pass: [kh;ones]x[qh;-max] + [kl;kh]x[qh;ql]) -> ACT exp(scale=1/8)
  causal mask via per-core bf16 lambda tiles (one SPMD program, all cores)
  AV: [vhi|ones] x E (+ [vlo|0] x E for self) -> pre rows + sum row
  1/sum via reciprocal + gpsimd partition_broadcast -> preT f32r -> wo f32r
  residual adds + feature-dim LN (ones-matmul stats + rank-1 A/B tiles)
"""
import os
import sys
import numpy as np

for _p in ("/opt/trn_rl_repo", "/root/.axon_site/_ro/trn_rl_repo"):
    if os.path.isdir(_p) and _p not in sys.path:
        sys.path.insert(0, _p)

import ml_dtypes
import concourse.bass as bass
import concourse.tile as tile
from concourse import bacc, mybir
from concourse.masks import make_identity

F32 = mybir.dt.float32
F32R = mybir.dt.float32r
BF16 = mybir.dt.bfloat16
AF = mybir.ActivationFunctionType
ALU = mybir.AluOpType
AX = mybir.AxisListType

B, C, M, H, K, V, F = 2, 2048, 1024, 16, 64, 64, 4096
EPS = 1e-5
SCALE = 1.0 / 8.0
MASKB = -100.0 * SCALE
NCORES = 8
CPB = NCORES // B
D = C // CPB
NH2 = H // 2
MT = M // 128
CT = C // 128
DT = D // 128
FT = F // 128
VA = V + 1
R4 = CPB  # ranks per replica group
RG = [[0, 1, 2, 3], [4, 5, 6, 7]]


def _proj3(nc, sbp, psp, wh_d, wl_d, xh_t, xl_t, mode, k_local=None,
           qst=None, qtag="qst"):
    """3-pass split-bf16 projection out = wh.T(xh+xl) + wl.T xh.

    xh_t/xl_t: lists of 8 [128, D] bf16 SBUF tiles (moving operands).
    mode "k": write k_local DRAM [128, H*D] rows [kl(0:64); kh(64:128)].
    mode "q": append per-head [qh(0:64); ql(64:128)] tiles to qst.
    """
    for jb in (0, 4):
        pos = [psp.tile([128, D], F32, tag="pacc", bufs=4, name="pjo")
               for _ in range(4)]
        for mt in range(MT):
            whc = sbp.tile([128, 512], BF16, tag="pw", bufs=6, name="whc")
            nc.sync.dma_start(
                out=whc[:],
                in_=wh_d.ap()[mt * 128:(mt + 1) * 128,
                              jb * 128:jb * 128 + 512])
            wlc = sbp.tile([128, 512], BF16, tag="pw", bufs=6, name="wlc")
            nc.sync.dma_start(
                out=wlc[:],
                in_=wl_d.ap()[mt * 128:(mt + 1) * 128,
                              jb * 128:jb * 128 + 512])
            for ji in range(4):
                wh = whc[:, ji * 128:(ji + 1) * 128]
                wl = wlc[:, ji * 128:(ji + 1) * 128]
                nc.tensor.matmul(pos[ji][:], wh, xh_t[mt][:],
                                 start=(mt == 0), stop=False)
                nc.tensor.matmul(pos[ji][:], wh, xl_t[mt][:],
                                 start=False, stop=False)
                nc.tensor.matmul(pos[ji][:], wl, xh_t[mt][:],
                                 start=False, stop=(mt == MT - 1))
        for ji in range(4):
            j = jb + ji
            if mode == "k":
                hi = sbp.tile([128, D], BF16, tag="kout", bufs=2, name="khi_o")
                nc.vector.tensor_copy(hi[:], pos[ji][:])
                lo = sbp.tile([128, D], BF16, tag="kout", bufs=2, name="klo_o")
                nc.vector.tensor_tensor(out=lo[:], in0=pos[ji][:], in1=hi[:],
                                        op=ALU.subtract)
                for s in range(2):
                    h = 2 * j + s
                    nc.sync.dma_start(
                        out=k_local[0:64, h * D:(h + 1) * D],
                        in_=lo[s * 64:(s + 1) * 64, :])
                    nc.sync.dma_start(
                        out=k_local[64:128, h * D:(h + 1) * D],
                        in_=hi[s * 64:(s + 1) * 64, :])
            else:
                hi = sbp.tile([128, D], BF16, tag="kout", bufs=2, name="qhi")
                nc.vector.tensor_copy(hi[:], pos[ji][:])
                lo = sbp.tile([128, D], BF16, tag="kout", bufs=2, name="qlo")
                nc.vector.tensor_tensor(out=lo[:], in0=pos[ji][:], in1=hi[:],
                                        op=ALU.subtract)
                for s in range(2):
                    q = sbp.tile([128, D], BF16, tag=qtag, bufs=16, name="q")
                    nc.vector.tensor_copy(q[0:64, :],
                                          hi[s * 64:(s + 1) * 64, :])
                    nc.vector.tensor_copy(q[64:128, :],
                                          lo[s * 64:(s + 1) * 64, :])
                    qst.append(q)


def _vproj3(nc, sbp, psp, wh_d, wl_d, xh_t, xl_t, v_local, split_out):
    """Token-major V projection (3-pass split).

    Writes v_local [128, ctl(4) (s) g4(4) hl(4) va(65)] bf16 with a ones
    column at va=64 in the hi half (zeros in the lo half when split_out).
    """
    GW = 4 * VA          # 260: one 4-head group incl. ones cols
    SW = 4 * GW          # 1040: all 4 groups for one (ctl, s)
    CW = (2 if split_out else 1) * SW
    for gp in range(2):
        for gi in range(2):
            g4 = gp * 2 + gi
            # one PSUM tile per chain: a start=True matmul clears its whole
            # bank, so accumulation chains must never share a bank
            pos = [psp.tile([128, 256], F32, tag="pacc", bufs=4, name="vpo")
                   for _ in range(4)]
            for mt in range(MT):
                whc = sbp.tile([128, 256], BF16, tag="pw", bufs=6,
                               name="vwh")
                nc.sync.dma_start(
                    out=whc[:],
                    in_=wh_d.ap()[mt * 128:(mt + 1) * 128,
                                  g4 * 256:(g4 + 1) * 256])
                wlc = sbp.tile([128, 256], BF16, tag="pw", bufs=6,
                               name="vwl")
                nc.sync.dma_start(
                    out=wlc[:],
                    in_=wl_d.ap()[mt * 128:(mt + 1) * 128,
                                  g4 * 256:(g4 + 1) * 256])
                for ctl in range(4):
                    sth = xh_t[mt][:, ctl * 128:(ctl + 1) * 128]
                    stl = xl_t[mt][:, ctl * 128:(ctl + 1) * 128]
                    out = pos[ctl][:]
                    nc.tensor.matmul(out, sth, whc[:], start=(mt == 0),
                                     stop=False)
                    nc.tensor.matmul(out, stl, whc[:], start=False,
                                     stop=False)
                    nc.tensor.matmul(out, sth, wlc[:], start=False,
                                     stop=(mt == MT - 1))
            for ctl in range(4):
                po = pos[ctl][:]
                po4 = po.rearrange("p (hl v) -> p hl v", v=64)
                vbh = sbp.tile([128, 4, VA], BF16, tag="vb", bufs=2,
                               name="vbh")
                nc.vector.tensor_copy(vbh[:, :, 0:64], po4)
                nc.vector.memset(vbh[:, :, 64:65], 1.0)
                nc.sync.dma_start(
                    out=v_local[:, ctl * CW + g4 * GW:
                                ctl * CW + g4 * GW + GW],
                    in_=vbh[:])
                if split_out:
                    vbl = sbp.tile([128, 4, VA], BF16, tag="vb", bufs=2,
                                   name="vbl")
                    nc.vector.tensor_tensor(out=vbl[:, :, 0:64], in0=po4,
                                            in1=vbh[:, :, 0:64],
                                            op=ALU.subtract)
                    nc.vector.memset(vbl[:, :, 64:65], 0.0)
                    nc.sync.dma_start(
                        out=v_local[:, ctl * CW + SW + g4 * GW:
                                    ctl * CW + SW + g4 * GW + GW],
                        in_=vbl[:])


def _layernorm(nc, sbp, psp, consts, r_tiles, g_d, b_d, split_out,
               ntag="n2"):
    """LN over the partition(feature) dim of 8 [128, D] f32r tiles."""
    ones128 = consts["ones128"]
    onesrow = consts["ones_row"]
    gvec = sbp.tile([1, M], F32R, tag="gbrow", bufs=2, name="gvec")
    nc.sync.dma_start(out=gvec[:], in_=g_d.ap())
    bvec = sbp.tile([1, M], F32R, tag="gbrow", bufs=2, name="bvec")
    nc.sync.dma_start(out=bvec[:], in_=b_d.ap())
    pss = psp.tile([1, D], F32, tag="pacc", bufs=4, name="pss")
    for mt in range(MT):
        nc.tensor.matmul(pss[:], ones128[:], r_tiles[mt][:],
                         start=(mt == 0), stop=(mt == MT - 1))
    psq = psp.tile([1, D], F32, tag="pacc", bufs=4, name="psq")
    for mt in range(MT):
        sq = sbp.tile([128, D], F32R, tag="lnsq", bufs=1, name="sq")
        nc.vector.tensor_tensor(out=sq[:], in0=r_tiles[mt][:],
                                in1=r_tiles[mt][:], op=ALU.mult)
        nc.tensor.matmul(psq[:], ones128[:], sq[:],
                         start=(mt == 0), stop=(mt == MT - 1))
    mu = sbp.tile([1, D], F32, tag="lnrow", bufs=4, name="mu")
    nc.vector.tensor_scalar_mul(mu[:], pss[:], 1.0 / M)
    var = sbp.tile([1, D], F32, tag="lnrow", bufs=4, name="var")
    nc.vector.tensor_scalar_mul(var[:], psq[:], 1.0 / M)
    mu2 = sbp.tile([1, D], F32, tag="lnrow", bufs=4, name="mu2")
    nc.vector.tensor_tensor(out=mu2[:], in0=mu[:], in1=mu[:], op=ALU.mult)
    nc.vector.tensor_tensor(out=var[:], in0=var[:], in1=mu2[:],
                            op=ALU.subtract)
    std = sbp.tile([1, D], F32, tag="lnrow", bufs=4, name="std")
    nc.scalar.activation(out=std[:], in_=var[:], func=AF.Sqrt,
                         bias=consts["eps1"][:])
    rstd = sbp.tile([1, D], F32R, tag="lnrowr", bufs=2, name="rstd")
    with nc.allow_low_precision(reason="f32r rounding for 1cyc/row matmul"):
        nc.vector.reciprocal(rstd[:], std[:])
    nmr = sbp.tile([1, D], F32R, tag="lnrowr", bufs=2, name="nmr")
    nc.vector.tensor_tensor(out=nmr[:], in0=mu[:], in1=rstd[:],
                            op=ALU.mult)
    nc.vector.tensor_scalar_mul(nmr[:], nmr[:], -1.0)

    outs = []
    for mt in range(MT):
        g = gvec[:, mt * 128:(mt + 1) * 128]
        b = bvec[:, mt * 128:(mt + 1) * 128]
        pa = psp.tile([128, D], F32, tag="pacc", bufs=4, name="pa")
        nc.tensor.matmul(pa[:], g, rstd[:], start=True, stop=True)
        pb = psp.tile([128, D], F32, tag="pacc", bufs=4, name="pb")
        nc.tensor.matmul(pb[:], b, onesrow[:], start=True, stop=False)
        nc.tensor.matmul(pb[:], g, nmr[:], start=False, stop=True)
        tmp = sbp.tile([128, D], F32, tag="lntmp", bufs=2, name="tmp")
        nc.vector.tensor_tensor(out=tmp[:], in0=r_tiles[mt][:],
                                in1=pa[:], op=ALU.mult)
        if split_out:
            n32 = sbp.tile([128, D], F32, tag="lnn32", bufs=2, name="n32")
            nc.vector.tensor_tensor(out=n32[:], in0=tmp[:], in1=pb[:],
                                    op=ALU.add)
            nh = sbp.tile([128, D], BF16, tag="n1", bufs=16, name="nh")
            nc.vector.tensor_copy(nh[:], n32[:])
            nl = sbp.tile([128, D], BF16, tag="n1", bufs=16, name="nl")
            nc.vector.tensor_tensor(out=nl[:], in0=n32[:], in1=nh[:],
                                    op=ALU.subtract)
            outs.append((nh, nl))
        else:
            n = sbp.tile([128, D], F32R, tag=ntag, bufs=8, name="n")
            nc.vector.tensor_tensor(out=n[:], in0=tmp[:], in1=pb[:],
                                    op=ALU.add)
            outs.append(n)
    return outs


def _attention(nc, sbp, psp, consts, qst, k_full, v_full, masked, vsplit,
               wo_d, xres_fn, g_d, b_d, mask_sb, ln_split, dbg_r=None,
               dbg_pt=None, dbg=None):
    """One attention block + residual + LN."""
    preT = [sbp.tile([128, D], F32R, tag="preT", bufs=8, name=f"preT{_j}")
            for _j in range(NH2)]
    kv = k_full.rearrange("(r p) (h d) -> p h r d", p=128, d=D)
    GW = 4 * VA
    G2W = 2 * VA
    nsl = 2 if vsplit else 1
    vv = v_full.rearrange("(r p) (ctl s w) -> p r ctl s w", p=128, s=nsl,
                          w=4 * GW)

    slab_h = slab_l = None
    for h in range(H):
        if h % 2 == 0:
            g2 = h // 2
            slab_h = sbp.tile([128, R4, 4, G2W], BF16, tag="vslab", bufs=4,
                              name="slab_h")
            for r in range(R4):
                nc.sync.dma_start(
                    out=slab_h[:, r],
                    in_=vv[:, r, :, 0, g2 * G2W:(g2 + 1) * G2W])
            if vsplit:
                slab_l = sbp.tile([128, R4, 4, G2W], BF16, tag="vslab",
                                  bufs=4, name="slab_l")
                for r in range(R4):
                    nc.sync.dma_start(
                        out=slab_l[:, r],
                        in_=vv[:, r, :, 1, g2 * G2W:(g2 + 1) * G2W])

        kst = sbp.tile([128, R4, D], BF16, tag="kst", bufs=4, name="kst")
        nc.sync.dma_start(out=kst[:], in_=kv[:, h])
        khi = sbp.tile([VA, R4, D], BF16, tag="khi", bufs=4, name="khi")
        nc.sync.dma_start(out=khi[0:64], in_=kv[64:128, h])
        nc.sync.dma_start(
            out=khi[64:65],
            in_=consts["ones2048"].ap().rearrange("o (r d) -> o r d", d=D))

        j2, o2 = h // 2, (h % 2) * K
        q = qst[h]
        if dbg is not None and h == 0:
            nc.sync.dma_start(
                out=dbg["dbgKST"].ap().rearrange("p (r d) -> p r d", d=D),
                in_=kst[:])
            nc.sync.dma_start(
                out=dbg["dbgKHI"].ap().rearrange("p (r d) -> p r d", d=D),
                in_=khi[:])
            nc.sync.dma_start(
                out=dbg["dbgSLB"].ap().rearrange(
                    "p (r ctl w) -> p r ctl w", r=R4, ctl=4),
                in_=slab_h[:])

        # QK-B on hi: negated max per d-tile
        psm = psp.tile([1, D], F32, tag="psm", bufs=2, name="psm")
        for dt in range(DT):
            nm = sbp.tile([128, 1], F32, tag="nmax", bufs=4, name="nm")
            for cc in range(4):
                psb = psp.tile([128, 512], F32, tag="pacc", bufs=4,
                               name="psb")
                nc.tensor.matmul(
                    psb[:], q[0:64, dt * 128:(dt + 1) * 128],
                    khi[0:64, cc, :], start=True, stop=True)
                if cc == 0:
                    nc.vector.reduce_max(nm[:], psb[:], axis=AX.X,
                                         negate=True)
                else:
                    nm2 = sbp.tile([128, 1], F32, tag="nmax2", bufs=2,
                                   name="nm2")
                    nc.vector.reduce_max(nm2[:], psb[:], axis=AX.X,
                                         negate=True)
                    nc.vector.tensor_tensor(out=nm[:], in0=nm[:], in1=nm2[:],
                                            op=ALU.min)
            nc.tensor.transpose(psm[0:1, dt * 128:(dt + 1) * 128], nm[:],
                                consts["identb"][:])
        rhi = sbp.tile([VA, D], BF16, tag="rhi", bufs=2, name="rhi")
        nc.vector.tensor_copy(rhi[0:64, :], q[0:64, :])
        nc.vector.tensor_copy(rhi[64:65, :], psm[:])
        if dbg is not None and h == 0:
            psmc = sbp.tile([1, D], F32, tag="rec", bufs=1, name="psmc")
            nc.vector.tensor_copy(psmc[:], psm[:])
            nc.sync.dma_start(out=dbg["dbgPSM"].ap(), in_=psmc[:])

        # QK-A (2 passes) + exp (+mask) + AV
        pp = psp.tile([VA, D], F32, tag="acc", bufs=2, name="pp")
        for ct in range(CT):
            r, cl = ct // 4, (ct % 4) * 128
            psa = psp.tile([128, D], F32, tag="pacc", bufs=4, name="psa")
            nc.tensor.matmul(psa[:], khi[:, r, cl:cl + 128], rhi[:],
                             start=True, stop=False)
            nc.tensor.matmul(psa[:], kst[:, r, cl:cl + 128], q[:],
                             start=False, stop=True)
            e = sbp.tile([128, D], BF16, tag="etile", bufs=3, name="e")
            nc.scalar.activation(out=e[:], in_=psa[:], func=AF.Exp,
                                 scale=SCALE)
            if masked:
                nc.vector.tensor_tensor(out=e[:], in0=e[:],
                                        in1=mask_sb[ct][:], op=ALU.mult)
            if dbg is not None and h == 0 and ct == 0:
                nc.sync.dma_start(out=dbg["dbgE0"].ap(), in_=e[:])
            hl = h % 2
            nc.tensor.matmul(pp[:], slab_h[:, r, ct % 4,
                                           hl * VA:(hl + 1) * VA], e[:],
                             start=(ct == 0),
                             stop=(not vsplit and ct == CT - 1))
            if vsplit:
                nc.tensor.matmul(pp[:], slab_l[:, r, ct % 4,
                                               hl * VA:(hl + 1) * VA], e[:],
                                 start=False, stop=(ct == CT - 1))

        rec = sbp.tile([1, D], F32, tag="rec", bufs=1, name="rec")
        nc.vector.reciprocal(rec[:], pp[64:65, :])
        rb = sbp.tile([K, D], F32, tag="rbcast", bufs=2, name="rb")
        nc.gpsimd.partition_broadcast(rb[:], rec[:])
        nc.vector.tensor_tensor(out=preT[j2][o2:o2 + K, :],
                                in0=pp[0:64, :], in1=rb[:], op=ALU.mult)

    # wo projection + residual
    if dbg_pt is not None:
        for j in range(NH2):
            nc.sync.dma_start(out=dbg_pt.ap()[j * 128:(j + 1) * 128, :],
                              in_=preT[j][:].bitcast(F32))
    r_tiles = []
    pos = []
    for mtg in range(2):
        pos = [psp.tile([128, D], F32, tag="pacc", bufs=4, name="wpo")
               for _ in range(4)]
        for j in range(NH2):
            woc = sbp.tile([128, 512], F32R, tag="pwr", bufs=4, name="woc")
            nc.sync.dma_start(
                out=woc[:],
                in_=wo_d.ap()[j * 128:(j + 1) * 128,
                              mtg * 512:(mtg + 1) * 512])
            for mi in range(4):
                nc.tensor.matmul(pos[mi][:], woc[:, mi * 128:(mi + 1) * 128],
                                 preT[j][:], start=(j == 0),
                                 stop=(j == NH2 - 1))
        for mi in range(4):
            mt = mtg * 4 + mi
            adds = xres_fn(mt)
            r = sbp.tile([128, D], F32R, tag="rres", bufs=8, name="r")
            nc.vector.tensor_tensor(out=r[:], in0=pos[mi][:],
                                    in1=adds[0][:], op=ALU.add)
            for extra in adds[1:]:
                nc.vector.tensor_tensor(out=r[:], in0=r[:],
                                        in1=extra[:], op=ALU.add)
            if dbg_r is not None:
                nc.sync.dma_start(
                    out=dbg_r.ap()[mt * 128:(mt + 1) * 128, :],
                    in_=r[:].bitcast(F32))
            r_tiles.append(r)

    n = _layernorm(nc, sbp, psp, consts, r_tiles, g_d, b_d, ln_split)
    return n


def build():
    nc = bacc.Bacc("TRN2", target_bir_lowering=False, debug=False,
                   num_devices=NCORES)
    inp = {}

    def di(name, shape, dt):
        inp[name] = nc.dram_tensor(name, shape, dt, kind="ExternalInput")
        return inp[name]

    for nm in ("xh", "xl", "eh", "el"):
        di(nm, [M, D], BF16)
    for w in ("wq1", "wk1", "wv1", "wq2", "wk2", "wv2"):
        di(w + "h", [M, M], BF16)
        di(w + "l", [M, M], BF16)
    di("wo1", [M, M], F32R)
    di("wo2", [M, M], F32R)
    di("fw1", [M, F], F32R)
    di("fw2", [F, M], BF16)
    for v in ("g1", "b1", "g2", "b2", "g3", "b3"):
        di(v, [1, M], F32R)
    di("fb1", [1, F], F32)
    di("fb2", [1, M], F32)
    di("ones2048", [1, C], BF16)
    di("ones128c", [128, 1], F32R)
    di("onesrowc", [1, D], F32R)
    di("maskT", [128, CT, D], BF16)
    outT = nc.dram_tensor("outT", [M, D], F32, kind="ExternalOutput")
    DBG = bool(os.environ.get("KDBG"))
    dbg = {}
    if DBG:
        for dn in ("dbgR1", "dbgN1", "dbgR2", "dbgN2", "dbgR3", "dbgPT"):
            dbg[dn] = nc.dram_tensor(dn, [M, D], F32, kind="ExternalOutput")
        dbg["dbgQ1"] = nc.dram_tensor("dbgQ1", [H * 128, D], BF16,
                                      kind="ExternalOutput")
        dbg["dbgKST"] = nc.dram_tensor("dbgKST", [128, R4 * D], BF16,
                                       kind="ExternalOutput")
        dbg["dbgKHI"] = nc.dram_tensor("dbgKHI", [VA, R4 * D], BF16,
                                       kind="ExternalOutput")
        dbg["dbgPSM"] = nc.dram_tensor("dbgPSM", [1, D], F32,
                                       kind="ExternalOutput")
        dbg["dbgE0"] = nc.dram_tensor("dbgE0", [128, D], BF16,
                                      kind="ExternalOutput")
        dbg["dbgSLB"] = nc.dram_tensor("dbgSLB", [128, R4 * 4 * 2 * VA],
                                       BF16, kind="ExternalOutput")

    with tile.TileContext(nc) as tc:
        import contextlib
        with contextlib.ExitStack() as ctx:
            sbp = ctx.enter_context(tc.tile_pool(name="sb", bufs=1))
            psp = ctx.enter_context(tc.tile_pool(name="ps", bufs=1,
                                                 space="PSUM"))
            sing = ctx.enter_context(tc.tile_pool(name="sing", bufs=1))
            dram = ctx.enter_context(tc.tile_pool(name="dram", bufs=1,
                                                  space="DRAM"))

            consts = {}
            identb = sing.tile([128, 128], F32)
            make_identity(nc, identb[:])
            consts["identb"] = identb
            ones128 = sing.tile([128, 1], F32R)
            nc.sync.dma_start(out=ones128[:], in_=inp["ones128c"].ap())
            consts["ones128"] = ones128
            onesrow = sing.tile([1, D], F32R)
            nc.sync.dma_start(out=onesrow[:], in_=inp["onesrowc"].ap())
            consts["ones_row"] = onesrow
            eps1 = sing.tile([1, 1], F32)
            nc.vector.memset(eps1[:], EPS)
            consts["eps1"] = eps1
            consts["ones2048"] = inp["ones2048"]
            fb1 = sing.tile([128, FT], F32)
            nc.sync.dma_start(
                out=fb1[:],
                in_=inp["fb1"].ap().rearrange("o (a b) -> o a b", b=128)[0]
                .rearrange("a b -> b a"))
            fb2c = sing.tile([128, MT], F32)
            nc.sync.dma_start(
                out=fb2c[:],
                in_=inp["fb2"].ap().rearrange("o (a b) -> o a b", b=128)[0]
                .rearrange("a b -> b a"))


            # preload x (transposed, split) once: reused by k1/v1/q1/resid
            xh_t, xl_t = [], []
            for mt in range(MT):
                xh = sbp.tile([128, D], BF16, tag="xt", bufs=16, name="xh")
                nc.sync.dma_start(
                    out=xh[:], in_=inp["xh"].ap()[mt * 128:(mt + 1) * 128, :])
                xl = sbp.tile([128, D], BF16, tag="xt", bufs=16, name="xl")
                nc.sync.dma_start(
                    out=xl[:], in_=inp["xl"].ap()[mt * 128:(mt + 1) * 128, :])
                xh_t.append(xh)
                xl_t.append(xl)
            eh_t, el_t = [], []
            for mt in range(MT):
                ehh = sbp.tile([128, D], BF16, tag="n1", bufs=16, name="ehh")
                nc.sync.dma_start(
                    out=ehh[:],
                    in_=inp["eh"].ap()[mt * 128:(mt + 1) * 128, :])
                ell = sbp.tile([128, D], BF16, tag="n1", bufs=16, name="ell")
                nc.sync.dma_start(
                    out=ell[:],
                    in_=inp["el"].ap()[mt * 128:(mt + 1) * 128, :])
                eh_t.append(ehh)
                el_t.append(ell)

            GW = 4 * VA
            k1_local = dram.tile([128, H * D], BF16)
            k1_full = dram.tile([R4 * 128, H * D], BF16)
            v1_local = dram.tile([128, 4 * 2 * 4 * GW], BF16)
            v1_full = dram.tile([R4 * 128, 4 * 2 * 4 * GW], BF16)
            k2_local = dram.tile([128, H * D], BF16)
            k2_full = dram.tile([R4 * 128, H * D], BF16)
            v2_local = dram.tile([128, 4 * 4 * GW], BF16)
            v2_full = dram.tile([R4 * 128, 4 * 4 * GW], BF16)

            _proj3(nc, sbp, psp, inp["wk1h"], inp["wk1l"], xh_t, xl_t, "k",
                   k_local=k1_local)
            _vproj3(nc, sbp, psp, inp["wv1h"], inp["wv1l"], xh_t, xl_t,
                    v1_local, True)
            nc.gpsimd.collective_compute(
                "AllGather", ALU.bypass, replica_groups=RG,
                ins=[k1_local.opt()], outs=[k1_full.opt()])
            nc.gpsimd.collective_compute(
                "AllGather", ALU.bypass, replica_groups=RG,
                ins=[v1_local.opt()], outs=[v1_full.opt()])
            _proj3(nc, sbp, psp, inp["wk2h"], inp["wk2l"], eh_t, el_t, "k",
                   k_local=k2_local)
            _vproj3(nc, sbp, psp, inp["wv2h"], inp["wv2l"], eh_t, el_t,
                    v2_local, False)
            nc.gpsimd.collective_compute(
                "AllGather", ALU.bypass, replica_groups=RG,
                ins=[k2_local.opt()], outs=[k2_full.opt()])
            nc.gpsimd.collective_compute(
                "AllGather", ALU.bypass, replica_groups=RG,
                ins=[v2_local.opt()], outs=[v2_full.opt()])

            # mask tiles ride the "n1" tag: e-tiles (k2/v2 proj inputs) are
            # dead by now, and LN1's outputs reclaim these slots afterwards
            mask_sb = []
            for ct in range(CT):
                mk = sbp.tile([128, D], BF16, tag="n1", bufs=16, name="mk")
                nc.sync.dma_start(out=mk[:], in_=inp["maskT"].ap()[:, ct, :])
                mask_sb.append(mk)

            qst1 = []
            _proj3(nc, sbp, psp, inp["wq1h"], inp["wq1l"], xh_t, xl_t, "q",
                   qst=qst1)
            if DBG:
                for h in range(H):
                    nc.sync.dma_start(
                        out=dbg["dbgQ1"].ap()[h * 128:(h + 1) * 128, :],
                        in_=qst1[h][:])

            def xres1(mt):
                return [xh_t[mt], xl_t[mt]]

            n1 = _attention(nc, sbp, psp, consts, qst1, k1_full, v1_full,
                            True, True, inp["wo1"], xres1, inp["g1"],
                            inp["b1"], mask_sb, ln_split=True,
                            dbg_r=dbg.get("dbgR1"), dbg_pt=dbg.get("dbgPT"),
                            dbg=(dbg if DBG else None))
            n1h = [t[0] for t in n1]
            n1l = [t[1] for t in n1]
            if DBG:
                for mt in range(MT):
                    n32d = sbp.tile([128, D], F32, tag="lntmp", bufs=2,
                                    name="n32d")
                    nc.vector.tensor_tensor(out=n32d[:], in0=n1h[mt][:],
                                            in1=n1l[mt][:], op=ALU.add)
                    nc.sync.dma_start(
                        out=dbg["dbgN1"].ap()[mt * 128:(mt + 1) * 128, :],
                        in_=n32d[:])

            qst2 = []
            _proj3(nc, sbp, psp, inp["wq2h"], inp["wq2l"], n1h, n1l, "q",
                   qst=qst2)

            def xres2(mt):
                return [n1h[mt], n1l[mt]]

            n2 = _attention(nc, sbp, psp, consts, qst2, k2_full, v2_full,
                            False, False, inp["wo2"], xres2, inp["g2"],
                            inp["b2"], None, ln_split=False,
                            dbg_r=dbg.get("dbgR2"))
            if DBG:
                for mt in range(MT):
                    nc.sync.dma_start(
                        out=dbg["dbgN2"].ap()[mt * 128:(mt + 1) * 128, :],
                        in_=n2[mt][:].bitcast(F32))

            # FFN
            h1 = []
            for ftg in range(8):
                pos = [psp.tile([128, D], F32, tag="pacc", bufs=4,
                                name="fpo") for _ in range(4)]
                for mt in range(MT):
                    w1c = sbp.tile([128, 512], F32R, tag="pwr", bufs=4,
                                   name="w1c")
                    nc.sync.dma_start(
                        out=w1c[:],
                        in_=inp["fw1"].ap()[mt * 128:(mt + 1) * 128,
                                            ftg * 512:(ftg + 1) * 512])
                    for fi in range(4):
                        nc.tensor.matmul(
                            pos[fi][:], w1c[:, fi * 128:(fi + 1) * 128],
                            n2[mt][:], start=(mt == 0), stop=(mt == MT - 1))
                for fi in range(4):
                    ft = ftg * 4 + fi
                    o = sbp.tile([128, D], BF16,
                                 tag=("xt" if ft < 16 else "n1"), bufs=16,
                                 name="o")
                    nc.scalar.activation(out=o[:], in_=pos[fi][:],
                                         func=AF.Relu,
                                         bias=fb1[:, ft:ft + 1])
                    h1.append(o)

            r3 = []
            for mtg in range(2):
                pos = [psp.tile([128, D], F32, tag="pacc", bufs=4,
                                name="fpo2") for _ in range(4)]
                for ft in range(FT):
                    w2c = sbp.tile([128, 512], BF16, tag="pw", bufs=6,
                                   name="w2c")
                    nc.sync.dma_start(
                        out=w2c[:],
                        in_=inp["fw2"].ap()[ft * 128:(ft + 1) * 128,
                                            mtg * 512:(mtg + 1) * 512])
                    for mi in range(4):
                        nc.tensor.matmul(
                            pos[mi][:], w2c[:, mi * 128:(mi + 1) * 128],
                            h1[ft][:], start=(ft == 0), stop=(ft == FT - 1))
                for mi in range(4):
                    mt = mtg * 4 + mi
                    t = sbp.tile([128, D], F32, tag="ffn_b", bufs=1,
                                 name="t")
                    nc.scalar.activation(out=t[:], in_=pos[mi][:],
                                         func=AF.Identity,
                                         bias=fb2c[:, mt:mt + 1], scale=1.0)
                    r = sbp.tile([128, D], F32R, tag="rres", bufs=8,
                                 name="r")
                    nc.vector.tensor_tensor(out=r[:], in0=t[:],
                                            in1=n2[mt][:], op=ALU.add)
                    if DBG:
                        nc.sync.dma_start(
                            out=dbg["dbgR3"].ap()[mt * 128:(mt + 1) * 128, :],
                            in_=r[:].bitcast(F32))
                    r3.append(r)

            n3 = _layernorm(nc, sbp, psp, consts, r3, inp["g3"], inp["b3"],
                            False, ntag="n2")
            for mt in range(MT):
                nc.sync.dma_start(out=outT.ap()[mt * 128:(mt + 1) * 128, :],
                                  in_=n3[mt][:].bitcast(F32))

    nc.compile()
    return nc


_CACHE = {}


def _get_nc():
    if "nc" not in _CACHE:
        _CACHE["nc"] = build()
    return _CACHE["nc"]


def _make_maskT(q):
    lam = np.exp(np.float32(MASKB))
    D0 = q * D
    i = np.arange(128)[:, None]
    j = np.arange(D)[None, :]
    m = np.ones((128, CT, D), np.float32)
    for ct in range(CT):
        m[:, ct, :] = np.where(D0 + j >= ct * 128 + i, lam, 1.0)
    return m.astype(ml_dtypes.bfloat16)


def _split(a):
    hi = a.astype(ml_dtypes.bfloat16)
    lo = (a - hi.astype(np.float32)).astype(ml_dtypes.bfloat16)
    return np.ascontiguousarray(hi), np.ascontiguousarray(lo)


def make_in_maps(inputs):
    x = np.asarray(inputs["x"], np.float32)
    enc = np.asarray(inputs["enc_out"], np.float32)

    def packw(w):  # [H, M, K] -> [M, H*K]
        return np.ascontiguousarray(
            np.asarray(w, np.float32).transpose(1, 0, 2).reshape(M, -1))

    base = {
        "wo1": np.ascontiguousarray(
            np.asarray(inputs["wo1"], np.float32).reshape(H * V, M)),
        "wo2": np.ascontiguousarray(
            np.asarray(inputs["wo2"], np.float32).reshape(H * V, M)),
        "fw1": np.ascontiguousarray(np.asarray(inputs["fw1"], np.float32)),
        "fw2": np.asarray(inputs["fw2"], np.float32).astype(
            ml_dtypes.bfloat16),
        "g1": np.asarray(inputs["g1"], np.float32).reshape(1, M),
        "b1": np.asarray(inputs["b1"], np.float32).reshape(1, M),
        "g2": np.asarray(inputs["g2"], np.float32).reshape(1, M),
        "b2": np.asarray(inputs["b2"], np.float32).reshape(1, M),
        "g3": np.asarray(inputs["g3"], np.float32).reshape(1, M),
        "b3": np.asarray(inputs["b3"], np.float32).reshape(1, M),
        "fb1": np.asarray(inputs["fb1"], np.float32).reshape(1, F),
        "fb2": np.asarray(inputs["fb2"], np.float32).reshape(1, M),
        "ones2048": np.ones((1, C), ml_dtypes.bfloat16),
        "ones128c": np.ones((128, 1), np.float32),
        "onesrowc": np.ones((1, D), np.float32),
    }
    for w in ("wq1", "wk1", "wv1", "wq2", "wk2", "wv2"):
        hi, lo = _split(packw(inputs[w]))
        base[w + "h"] = hi
        base[w + "l"] = lo
    masks = [_make_maskT(q) for q in range(CPB)]
    in_maps = []
    for core in range(NCORES):
        b, q = core // CPB, core % CPB
        m = dict(base)
        xh, xl = _split(np.ascontiguousarray(x[b, q * D:(q + 1) * D, :].T))
        eh, el = _split(np.ascontiguousarray(enc[b, q * D:(q + 1) * D, :].T))
        m["xh"], m["xl"], m["eh"], m["el"] = xh, xl, eh, el
        m["maskT"] = masks[q]
        in_maps.append(m)
    return in_maps


def run_spmd(inputs, **kw):
    from concourse.bass_utils import run_bass_kernel_spmd
    nc = _get_nc()
    in_maps = make_in_maps(inputs)
    res = run_bass_kernel_spmd(nc, in_maps, core_ids=list(range(NCORES)), **kw)
    out = np.empty((B, C, M), np.float32)
    for core in range(NCORES):
        b, q = core // CPB, core % CPB
        out[b, q * D:(q + 1) * D, :] = res.results[core]["outT"].T
    return out, res


def kernel(**inputs):
    out, _ = run_spmd(inputs)
    return out
